# revision 5
# baseline (speedup 1.0000x reference)
"""Continual-attention Trainium2 kernel (8 NeuronCores, SPMD).

Sharding: core c -> batch b = c//2, head-group g = c%2 (4 heads each).
Per (b,h) computes S^T[k,q] = K Q^T via PE (float32r), additive masks
(causal diag / per-batch test-train / test-chunk) accumulated into PSUM
with extra matmuls, exp on ScalarE with fused 1/sqrt(d) scale, then
O^T[d,q] (+ softmax denominator as a 65th row via a ones column in V)
accumulated on PE. Normalization + final transpose happen on host.
"""

import sys

sys.path.insert(0, "/opt/trn_rl_repo")

import numpy as np
import ml_dtypes

B, L, H, D = 4, 2048, 8, 64
TRAIN = 1536
TEST = L - TRAIN            # 512
NCH = 64                    # test chunks
CH = TEST // NCH            # 8
HPC = 4                     # heads per core
NCORES = 8
KT = L // 128               # 16 k-tiles
NEG = -60000.0  # exp(NEG*0.125) == 0; fits fp16

LAST_RESULT = None          # BassKernelResults of the most recent run
_PROG = None                # cached compiled Bass program


def _split_multi_waits(nc, mybir):
    """This container's walrus accepts at most one semaphore wait per
    instruction; Tile's tail drains can carry several. Hoist extras onto
    NoOps inserted immediately before, on the same engine."""
    for f in nc.m.functions:
        for bb in f.blocks:
            insts = list(bb.instructions)
            out = []
            changed = False
            for inst in insts:
                si = inst.sync_info
                if si is not None and len(si.on_wait) > 1:
                    waits = list(si.on_wait)
                    for w in waits[:-1]:
                        nop = mybir.InstNoOp(
                            name=f"waitnop-{nc.next_id()}", ins=[], outs=[]
                        )
                        nop.engine = inst.engine
                        nop.sync_info = mybir.SyncInfo(on_wait=[w], on_update=[])
                        out.append(nop)
                    inst.sync_info = mybir.SyncInfo(
                        on_wait=[waits[-1]], on_update=list(si.on_update)
                    )
                    changed = True
                out.append(inst)
            if changed:
                bb.instructions = out


def _build_program():
    import concourse.bass as bass
    import concourse.mybir as mybir
    import concourse.tile as tile

    f32 = mybir.dt.float32
    bf16 = mybir.dt.bfloat16
    f32r = mybir.dt.float32r
    fp16 = mybir.dt.float16
    Exp = mybir.ActivationFunctionType.Exp

    nc = bass.Bass()

    qt_d = nc.dram_tensor("qt", [HPC, 128, L], fp16, kind="ExternalInput")
    kt_d = nc.dram_tensor("kt", [HPC, 128, L], fp16, kind="ExternalInput")
    vw_d = nc.dram_tensor("vw", [HPC, 128, KT * 65], fp16, kind="ExternalInput")
    mtt_d = nc.dram_tensor("mtt", [128, 12 * 512], fp16, kind="ExternalInput")
    ident_d = nc.dram_tensor("ident", [128, 128], fp16, kind="ExternalInput")
    mdiag_d = nc.dram_tensor("mdiag", [128, 128], fp16, kind="ExternalInput")
    mchunk_d = nc.dram_tensor("mchunk", [128, 128], fp16, kind="ExternalInput")
    ot_d = nc.dram_tensor("ot", [HPC, 65, L], f32, kind="ExternalOutput")

    with tile.TileContext(nc) as tc:
        with (
            tc.tile_pool(name="consts", bufs=1) as consts,
            tc.tile_pool(name="heads", bufs=2) as heads,
            tc.tile_pool(name="ptp", bufs=8) as ptp,
            tc.tile_pool(name="osbp", bufs=3) as osbp,
            tc.tile_pool(name="spp", bufs=2, space="PSUM") as spp,
            tc.tile_pool(name="avp", bufs=2, space="PSUM") as avp,
        ):
            ident_sb = consts.tile([128, 128], fp16)
            nc.sync.dma_start(out=ident_sb, in_=ident_d.ap())
            mdiag_sb = consts.tile([128, 128], fp16)
            nc.sync.dma_start(out=mdiag_sb, in_=mdiag_d.ap())
            mchunk_sb = consts.tile([128, 128], fp16)
            nc.sync.dma_start(out=mchunk_sb, in_=mchunk_d.ap())
            mtt_sb = consts.tile([128, 12 * 512], fp16)

            first = True
            for h in range(HPC):
                # qt/kt are zero-padded to 128 contraction rows: K=64 matmuls
                # never leave the PE's throttled clock state (HW-measured
                # 430ns vs 216ns per 512-col matmul), K=128 ones do.
                # DMAs are split into chunks so head-0 compute can start as
                # soon as its first k/q columns land.
                qt_sb = heads.tile([128, L], fp16, tag="qt")
                kt_sb = heads.tile([128, L], fp16, tag="kt")
                vw_sb = heads.tile([128, KT, 65], fp16, tag="vw")
                nch = 4 if first else 1
                step = L // nch
                for i in range(nch):
                    sl = slice(i * step, (i + 1) * step)
                    nc.sync.dma_start(out=kt_sb[:, sl], in_=kt_d.ap()[h][:, sl])
                for i in range(nch):
                    sl = slice(i * step, (i + 1) * step)
                    nc.sync.dma_start(out=qt_sb[:, sl], in_=qt_d.ap()[h][:, sl])
                nc.sync.dma_start(
                    out=vw_sb,
                    in_=vw_d.ap()[h].rearrange("p (t c) -> p t c", t=KT),
                )
                if first:
                    # per-batch test-train 0/1 mask, only needed from gq3 on
                    nc.sync.dma_start(out=mtt_sb, in_=mtt_d.ap())
                    first = False

                for gq in range(4):
                    av = avp.tile([128, 512], f32, tag="av")
                    kps = list(range(4 * (gq + 1))) if gq < 3 else list(range(16))
                    last_kp = kps[-1]

                    # chunk geometry per kp: (off, w) in q-group coords
                    def geom(kp):
                        if kp <= 11:
                            off = max(0, 128 * kp - 512 * gq)
                            return off, 512 - off
                        off = 128 * (kp - 12)
                        return off, 128

                    # Steps are processed in PAIRS sharing one 2-bank PSUM
                    # tile and ONE exp op with an exact span: chunk A at
                    # [0:wa], chunk B packed at [wa:wa+wb] if it fits in
                    # bank 0, else at [512:512+wb].
                    PIPE = 3  # pairs of AV matmuls held back
                    pending = []

                    def emit_av(kp, pt, pos, off, w, start, stop):
                        nc.tensor.matmul(
                            av[:65, off : off + w],
                            lhsT=vw_sb[:, kp, :],
                            rhs=pt[:, pos : pos + w],
                            start=start,
                            stop=stop,
                            skip_group_check=True,
                        )

                    def emit_s(kp, sp2, pos, off, w):
                        qs = 512 * gq + off
                        nc.tensor.matmul(
                            sp2[:, pos : pos + w],
                            lhsT=kt_sb[:, 128 * kp : 128 * kp + 128],
                            rhs=qt_sb[:, qs : qs + w],
                            start=True,
                            stop=True,
                            skip_group_check=True,
                        )

                    for i in range(0, len(kps), 2):
                        ka, kb = kps[i], kps[i + 1]
                        offa, wa = geom(ka)
                        offb, wb = geom(kb)
                        posa = 0
                        posb = wa if wa + wb <= 512 else 512
                        span = posb + wb

                        sp2 = spp.tile([128, 1024], f32, tag="sp")
                        emit_s(ka, sp2, posa, offa, wa)
                        emit_s(kb, sp2, posb, offb, wb)

                        pt = ptp.tile([128, 1024], fp16, tag="pt")
                        nc.scalar.activation(
                            pt[:, 0:span], sp2[:, 0:span], Exp, scale=0.125
                        )
                        if gq == 3 and ka <= 11:
                            # per-batch test-train mask: 0/1 multiply on DVE
                            # (pair spans cols [512*ka, 512*kb+512) of mtt)
                            nc.vector.tensor_mul(
                                pt[:, 0:1024],
                                pt[:, 0:1024],
                                mtt_sb[:, 512 * ka : 512 * ka + 1024],
                            )
                        for kp, pos in ((ka, posa), (kb, posb)):
                            if kp <= 11 and 128 * kp >= 512 * gq:
                                nc.vector.tensor_mul(
                                    pt[:, pos : pos + 128],
                                    pt[:, pos : pos + 128],
                                    mdiag_sb,
                                )
                            elif kp >= 12:
                                nc.vector.tensor_mul(
                                    pt[:, pos : pos + 128],
                                    pt[:, pos : pos + 128],
                                    mchunk_sb,
                                )

                        pending.append(
                            (
                                (ka, pt, posa, offa, wa, ka == 0, ka == last_kp),
                                (kb, pt, posb, offb, wb, kb == 0, kb == last_kp),
                            )
                        )
                        if len(pending) > PIPE:
                            for args in pending.pop(0):
                                emit_av(*args)

                    for pair in pending:
                        for args in pair:
                            emit_av(*args)

                    osb = osbp.tile([65, 512], f32)
                    nc.vector.tensor_copy(osb, av[:65, :])
                    nc.sync.dma_start(
                        out=ot_d.ap()[h][:, 512 * gq : 512 * gq + 512], in_=osb
                    )

    import concourse.mybir as mybir_mod

    _split_multi_waits(nc, mybir_mod)
    return nc


def _host_inputs(queries, keys, values, attach):
    """Build per-core input maps (host-side layout prep)."""
    f16 = np.float16
    p = np.arange(128)
    f = np.arange(128)
    ident = (p[:, None] == f[None, :]).astype(np.float32)
    mdiag = np.where(f[None, :] >= p[:, None], 1.0, 0.0).astype(np.float32)
    mchunk = np.where(
        (p[:, None] // CH == f[None, :] // CH) & (p[:, None] <= f[None, :]),
        1.0,
        0.0,
    ).astype(np.float32)

    in_maps = []
    for c in range(NCORES):
        b, g = divmod(c, 2)
        hs = slice(HPC * g, HPC * (g + 1))
        q = queries[b][:, hs, :]          # [L, 4, D]
        k = keys[b][:, hs, :]
        v = values[b][:, hs, :]
        qt = np.zeros((HPC, 128, L), np.float32)
        qt[:, :D, :] = q.transpose(1, 2, 0)
        kt = np.zeros((HPC, 128, L), np.float32)
        kt[:, :D, :] = k.transpose(1, 2, 0)
        vw = np.empty((HPC, L, 65), np.float32)
        vw[:, :, :64] = v.transpose(1, 0, 2)
        vw[:, :, 64] = 1.0
        # [4, L, 65] -> [4, 128, KT*65] with row p holding tile-chunks
        vw = np.ascontiguousarray(
            vw.reshape(HPC, KT, 128, 65).transpose(0, 2, 1, 3).reshape(HPC, 128, KT * 65)
        )
        kg = (np.arange(12)[:, None] * 128 + np.arange(128)[None, :])  # [12,128]
        thr = attach[b][np.arange(TEST) // CH]                          # [512]
        mtt = np.where(kg[:, :, None] <= thr[None, None, :], 1.0, 0.0)  # [12,128,512]
        mtt = np.ascontiguousarray(mtt.transpose(1, 0, 2).reshape(128, 12 * 512))
        in_maps.append(
            {
                "qt": qt.astype(f16),
                "kt": kt.astype(f16),
                "vw": vw.astype(f16),
                "mtt": mtt.astype(f16),
                "ident": ident.astype(f16),
                "mdiag": mdiag.astype(f16),
                "mchunk": mchunk.astype(f16),
            }
        )
    return in_maps


def kernel(queries, keys, values, attach_test_after, train_len):
    global LAST_RESULT, _PROG
    import os

    queries = np.asarray(queries, dtype=np.float32)
    keys = np.asarray(keys, dtype=np.float32)
    values = np.asarray(values, dtype=np.float32)
    attach = np.asarray(attach_test_after).astype(np.int64)
    tl = int(np.asarray(train_len))
    assert queries.shape == (B, L, H, D), queries.shape
    assert tl == TRAIN and attach.shape == (B, NCH)

    from concourse.bass_utils import run_bass_kernel_spmd

    if _PROG is None:
        _PROG = _build_program()

    in_maps = _host_inputs(queries, keys, values, attach)
    trace = bool(int(os.environ.get("KERNEL_TRACE", "0")))
    res = run_bass_kernel_spmd(
        _PROG, in_maps, core_ids=list(range(NCORES)), trace=trace
    )
    LAST_RESULT = res

    out = np.empty((B, L, H * D), np.float32)
    for c in range(NCORES):
        b, g = divmod(c, 2)
        ot = res.results[c]["ot"]                     # [4, 65, L]
        o = ot[:, :64, :] / ot[:, 64:65, :]           # [4, 64, L]
        out[b, :, 256 * g : 256 * (g + 1)] = (
            o.transpose(2, 0, 1).reshape(L, HPC * D)
        )
    return out



# revision 9
# speedup vs baseline: 15452.3542x; 15452.3542x over previous
"""Continual-attention Trainium2 kernel (8 NeuronCores, SPMD).

Sharding: core c -> batch b = c//2, head-group g = c%2 (4 heads each).
Per (b,h) computes S^T[k,q] = K Q^T via PE with K=64 contraction row-tiled
onto alternating halves of the PE array (2x concurrency), exp on ScalarE
over groups of 3 k-tiles (1536-col PSUM spans), multiplicative 0/1 masks
on DVE, then O^T[65,q] (64 dims + denominator row via ones column in V)
accumulated on PE. Normalization + final transpose happen on host.
"""

import sys

sys.path.insert(0, "/opt/trn_rl_repo")

import numpy as np

B, L, H, D = 4, 2048, 8, 64
TRAIN = 1536
TEST = L - TRAIN            # 512
NCH = 64                    # test chunks
CH = TEST // NCH            # 8
HPC = 4                     # heads per core
NCORES = 8
KT = L // 128               # 16 k-tiles

KQV = 2 * L + KT * 65       # combined per-head columns: kt | qt | vw
QOFF = L                    # qt column offset in kqv
VOFF = 2 * L                # vw column offset in kqv

GROUP = 3                   # k-tiles per PSUM tile / EXP instruction
SPW = 512 * GROUP           # PSUM tile width
PIPE = 2                    # groups of AV matmuls held back

LAST_RESULT = None          # BassKernelResults of the most recent run
_PROG = None                # cached compiled Bass program


def _split_multi_waits(nc, mybir):
    """This container's walrus accepts at most one semaphore wait per
    instruction; Tile's tail drains can carry several. Hoist extras onto
    NoOps inserted immediately before, on the same engine."""
    for f in nc.m.functions:
        for bb in f.blocks:
            insts = list(bb.instructions)
            out = []
            changed = False
            for inst in insts:
                si = inst.sync_info
                if si is not None and len(si.on_wait) > 1:
                    waits = list(si.on_wait)
                    for w in waits[:-1]:
                        nop = mybir.InstNoOp(
                            name=f"waitnop-{nc.next_id()}", ins=[], outs=[]
                        )
                        nop.engine = inst.engine
                        nop.sync_info = mybir.SyncInfo(on_wait=[w], on_update=[])
                        out.append(nop)
                    inst.sync_info = mybir.SyncInfo(
                        on_wait=[waits[-1]], on_update=list(si.on_update)
                    )
                    changed = True
                out.append(inst)
            if changed:
                bb.instructions = out


def _geom(kp, gq):
    """(off, w) of k-tile kp's q-span inside q-group gq (512 wide)."""
    if kp <= 11:
        off = max(0, 128 * kp - 512 * gq)
        return off, 512 - off
    off = 128 * (kp - 12)
    return off, 128


def _groups(gq):
    """Pack this gq's k-tiles into groups of <=GROUP with bank-aligned
    positions (no matmul output crosses a 512-col PSUM bank boundary).
    Returns list of [(kp, pos, off, w), ...] per group."""
    kps = list(range(4 * (gq + 1))) if gq < 3 else list(range(16))
    out = []
    cur = []
    pos = 0
    for kp in kps:
        off, w = _geom(kp, gq)
        # bank-align: place within current 512 bank if it fits
        bank_rem = -pos % 512
        if bank_rem and w > bank_rem:
            pos += bank_rem
        if len(cur) == GROUP or pos + w > SPW:
            out.append(cur)
            cur = []
            pos = 0
        cur.append((kp, pos, off, w))
        pos += w
    if cur:
        out.append(cur)
    return out


def _build_program():
    import os
    import concourse.bass as bass
    import concourse.mybir as mybir
    import concourse.tile as tile

    rowtile = bool(int(os.environ.get("K_ROWTILE", "1")))
    scalar_dma = bool(int(os.environ.get("K_SCALAR_DMA", "1")))

    f32 = mybir.dt.float32
    fp16 = mybir.dt.float16
    Exp = mybir.ActivationFunctionType.Exp

    nc = bass.Bass()

    kqv_d = nc.dram_tensor("kqv", [HPC, 128, KQV], fp16, kind="ExternalInput")
    mtt_d = nc.dram_tensor("mtt", [128, 12 * 512], fp16, kind="ExternalInput")
    msk_d = nc.dram_tensor("msk", [128, 256], fp16, kind="ExternalInput")
    ot_d = nc.dram_tensor("ot", [HPC, 65, L], f32, kind="ExternalOutput")

    with tile.TileContext(nc) as tc:
        with (
            tc.tile_pool(name="consts", bufs=1) as consts,
            tc.tile_pool(name="heads", bufs=4) as heads,
            tc.tile_pool(name="ptp", bufs=4) as ptp,
            tc.tile_pool(name="osbp", bufs=3) as osbp,
            tc.tile_pool(name="spp", bufs=2, space="PSUM") as spp,
            tc.tile_pool(name="avp", bufs=2, space="PSUM") as avp,
        ):
            # ---- input DMAs, all issued up front --------------------------
            msk_sb = consts.tile([128, 256], fp16)
            mtt_sb = consts.tile([128, 12 * 512], fp16)
            kqv_sbs = []
            for h in range(HPC):
                kqv_sbs.append(
                    heads.tile([128, KQV], fp16, tag="kqv", name=f"kqv{h}")
                )

            # head 0 split for early start: kt/qt halves, scalar-queue twin
            eng2 = nc.scalar if scalar_dma else nc.sync
            eng2.dma_start(out=kqv_sbs[0][:, 0:1024], in_=kqv_d.ap()[0][:, 0:1024])
            nc.sync.dma_start(
                out=kqv_sbs[0][:, QOFF : QOFF + 1024],
                in_=kqv_d.ap()[0][:, QOFF : QOFF + 1024],
            )
            eng2.dma_start(out=msk_sb, in_=msk_d.ap())
            nc.sync.dma_start(
                out=kqv_sbs[0][:, 1024:2048], in_=kqv_d.ap()[0][:, 1024:2048]
            )
            nc.sync.dma_start(
                out=kqv_sbs[0][:, QOFF + 1024 : VOFF],
                in_=kqv_d.ap()[0][:, QOFF + 1024 : VOFF],
            )
            nc.sync.dma_start(
                out=kqv_sbs[0][:, VOFF:KQV], in_=kqv_d.ap()[0][:, VOFF:KQV]
            )
            nc.sync.dma_start(out=kqv_sbs[1], in_=kqv_d.ap()[1])
            nc.sync.dma_start(out=mtt_sb, in_=mtt_d.ap())
            nc.sync.dma_start(out=kqv_sbs[2], in_=kqv_d.ap()[2])
            nc.sync.dma_start(out=kqv_sbs[3], in_=kqv_d.ap()[3])

            mdiag = msk_sb[:, 0:128]
            mchunk = msk_sb[:, 128:256]

            s_idx = 0  # global S-matmul counter for PE row-half alternation
            for h in range(HPC):
                kqv_sb = kqv_sbs[h]
                for gq in range(4):
                    av = avp.tile([128, 512], f32, tag="av")
                    groups = _groups(gq)
                    last_kp = groups[-1][-1][0]
                    pending = []

                    for grp in groups:
                        span = grp[-1][1] + grp[-1][3]
                        sp = spp.tile([128, SPW], f32, tag="sp")
                        for kp, pos, off, w in grp:
                            if rowtile:
                                half = (
                                    slice(0, 64) if s_idx % 2 == 0 else slice(64, 128)
                                )
                            else:
                                # K/Q rows are duplicated, so a full-128
                                # contraction computes 2*S; EXP scale halves.
                                half = slice(0, 128)
                            s_idx += 1
                            qs = QOFF + 512 * gq + off
                            nc.tensor.matmul(
                                sp[:, pos : pos + w],
                                lhsT=kqv_sb[half, 128 * kp : 128 * kp + 128],
                                rhs=kqv_sb[half, qs : qs + w],
                                start=True,
                                stop=True,
                                skip_group_check=True,
                            )
                        pt = ptp.tile([128, SPW], fp16, tag="pt")
                        nc.scalar.activation(
                            pt[:, 0:span],
                            sp[:, 0:span],
                            Exp,
                            scale=0.125 if rowtile else 0.0625,
                        )
                        if gq == 3 and grp[0][0] <= 11:
                            # per-batch test-train 0/1 mask on DVE; group spans
                            # mtt cols [512*kp0, 512*kp0 + span)
                            m0 = 512 * grp[0][0]
                            nc.vector.tensor_mul(
                                pt[:, 0:span],
                                pt[:, 0:span],
                                mtt_sb[:, m0 : m0 + span],
                            )
                        for kp, pos, off, w in grp:
                            if kp <= 11 and 128 * kp >= 512 * gq:
                                nc.vector.tensor_mul(
                                    pt[:, pos : pos + 128],
                                    pt[:, pos : pos + 128],
                                    mdiag,
                                )
                            elif kp >= 12:
                                nc.vector.tensor_mul(
                                    pt[:, pos : pos + 128],
                                    pt[:, pos : pos + 128],
                                    mchunk,
                                )

                        pending.append((grp, pt))
                        if len(pending) > PIPE:
                            pgrp, ppt = pending.pop(0)
                            for kp, pos, off, w in pgrp:
                                nc.tensor.matmul(
                                    av[:65, off : off + w],
                                    lhsT=kqv_sb[:, VOFF + 65 * kp : VOFF + 65 * kp + 65],
                                    rhs=ppt[:, pos : pos + w],
                                    start=kp == 0,
                                    stop=kp == last_kp,
                                    skip_group_check=True,
                                )

                    for pgrp, ppt in pending:
                        for kp, pos, off, w in pgrp:
                            nc.tensor.matmul(
                                av[:65, off : off + w],
                                lhsT=kqv_sb[:, VOFF + 65 * kp : VOFF + 65 * kp + 65],
                                rhs=ppt[:, pos : pos + w],
                                start=kp == 0,
                                stop=kp == last_kp,
                                skip_group_check=True,
                            )

                    osb = osbp.tile([65, 512], f32)
                    nc.vector.tensor_copy(osb, av[:65, :])
                    nc.sync.dma_start(
                        out=ot_d.ap()[h][:, 512 * gq : 512 * gq + 512], in_=osb
                    )

    import concourse.mybir as mybir_mod

    _split_multi_waits(nc, mybir_mod)
    return nc


def _host_inputs(queries, keys, values, attach):
    """Build per-core input maps (host-side layout prep)."""
    f16 = np.float16
    p = np.arange(128)
    f = np.arange(128)
    mdiag = np.where(f[None, :] >= p[:, None], 1.0, 0.0).astype(np.float32)
    mchunk = np.where(
        (p[:, None] // CH == f[None, :] // CH) & (p[:, None] <= f[None, :]),
        1.0,
        0.0,
    ).astype(np.float32)
    msk = np.concatenate([mdiag, mchunk], axis=1)  # [128, 256]

    in_maps = []
    for c in range(NCORES):
        b, g = divmod(c, 2)
        hs = slice(HPC * g, HPC * (g + 1))
        q = queries[b][:, hs, :]          # [L, 4, D]
        k = keys[b][:, hs, :]
        v = values[b][:, hs, :]
        qt = q.transpose(1, 2, 0)         # [4, 64, L]
        kt = k.transpose(1, 2, 0)
        vw = np.empty((HPC, L, 65), np.float32)
        vw[:, :, :64] = v.transpose(1, 0, 2)
        vw[:, :, 64] = 1.0
        # [4, L, 65] -> [4, 128, KT*65] with row p holding tile-chunks
        vw = np.ascontiguousarray(
            vw.reshape(HPC, KT, 128, 65).transpose(0, 2, 1, 3).reshape(HPC, 128, KT * 65)
        )
        # combined [4, 128, KQV]: kt | qt | vw, with K/Q duplicated into
        # partitions 64-127 for PE row-tiling
        kqv = np.empty((HPC, 128, KQV), np.float32)
        kqv[:, :64, 0:L] = kt
        kqv[:, 64:, 0:L] = kt
        kqv[:, :64, QOFF:VOFF] = qt
        kqv[:, 64:, QOFF:VOFF] = qt
        kqv[:, :, VOFF:] = vw
        kg = (np.arange(12)[:, None] * 128 + np.arange(128)[None, :])  # [12,128]
        thr = attach[b][np.arange(TEST) // CH]                          # [512]
        mtt = np.where(kg[:, :, None] <= thr[None, None, :], 1.0, 0.0)  # [12,128,512]
        mtt = np.ascontiguousarray(mtt.transpose(1, 0, 2).reshape(128, 12 * 512))
        in_maps.append(
            {
                "kqv": kqv.astype(f16),
                "mtt": mtt.astype(f16),
                "msk": msk.astype(f16),
            }
        )
    return in_maps


def kernel(queries, keys, values, attach_test_after, train_len):
    global LAST_RESULT, _PROG
    import os

    queries = np.asarray(queries, dtype=np.float32)
    keys = np.asarray(keys, dtype=np.float32)
    values = np.asarray(values, dtype=np.float32)
    attach = np.asarray(attach_test_after).astype(np.int64)
    tl = int(np.asarray(train_len))
    assert queries.shape == (B, L, H, D), queries.shape
    assert tl == TRAIN and attach.shape == (B, NCH)

    from concourse.bass_utils import run_bass_kernel_spmd

    if _PROG is None:
        _PROG = _build_program()

    in_maps = _host_inputs(queries, keys, values, attach)
    trace = bool(int(os.environ.get("KERNEL_TRACE", "0")))
    res = run_bass_kernel_spmd(
        _PROG, in_maps, core_ids=list(range(NCORES)), trace=trace
    )
    LAST_RESULT = res

    out = np.empty((B, L, H * D), np.float32)
    for c in range(NCORES):
        b, g = divmod(c, 2)
        ot = res.results[c]["ot"]                     # [4, 65, L]
        o = ot[:, :64, :] / ot[:, 64:65, :]           # [4, 64, L]
        out[b, :, 256 * g : 256 * (g + 1)] = (
            o.transpose(2, 0, 1).reshape(L, HPC * D)
        )
    return out


# revision 14
# speedup vs baseline: 16276.0343x; 1.0533x over previous
"""Continual-attention Trainium2 kernel (8 NeuronCores, SPMD).

Sharding: core c -> batch b = c//2, head-group g = c%2 (4 heads each).
Per (b,h) computes S^T[k,q] = K Q^T via PE with K=64 contraction row-tiled
onto alternating halves of the PE array (2x concurrency), exp on ScalarE
over groups of 3 k-tiles (1536-col PSUM spans), multiplicative 0/1 masks
on DVE, then O^T[65,q] (64 dims + denominator row via ones column in V)
accumulated on PE. Normalization + final transpose happen on host.
"""

import sys

sys.path.insert(0, "/opt/trn_rl_repo")

import numpy as np

B, L, H, D = 4, 2048, 8, 64
TRAIN = 1536
TEST = L - TRAIN            # 512
NCH = 64                    # test chunks
CH = TEST // NCH            # 8
HPC = 4                     # heads per core
NCORES = 8
KT = L // 128               # 16 k-tiles

KQV = 2 * L + KT * 65       # combined per-head columns: kt | qt | vw
QOFF = L                    # qt column offset in kqv
VOFF = 2 * L                # vw column offset in kqv

GROUP = 3                   # k-tiles per PSUM tile / EXP instruction
SPW = 512 * GROUP           # PSUM tile width
PIPE = 2                    # groups of AV matmuls held back

LAST_RESULT = None          # BassKernelResults of the most recent run
_PROG = None                # cached compiled Bass program


def _split_multi_waits(nc, mybir):
    """This container's walrus accepts at most one semaphore wait per
    instruction; Tile's tail drains can carry several. Hoist extras onto
    NoOps inserted immediately before, on the same engine."""
    for f in nc.m.functions:
        for bb in f.blocks:
            insts = list(bb.instructions)
            out = []
            changed = False
            for inst in insts:
                si = inst.sync_info
                if si is not None and len(si.on_wait) > 1:
                    waits = list(si.on_wait)
                    for w in waits[:-1]:
                        nop = mybir.InstNoOp(
                            name=f"waitnop-{nc.next_id()}", ins=[], outs=[]
                        )
                        nop.engine = inst.engine
                        nop.sync_info = mybir.SyncInfo(on_wait=[w], on_update=[])
                        out.append(nop)
                    inst.sync_info = mybir.SyncInfo(
                        on_wait=[waits[-1]], on_update=list(si.on_update)
                    )
                    changed = True
                out.append(inst)
            if changed:
                bb.instructions = out


def _geom(kp, gq):
    """(off, w) of k-tile kp's q-span inside q-group gq (512 wide)."""
    if kp <= 11:
        off = max(0, 128 * kp - 512 * gq)
        return off, 512 - off
    off = 128 * (kp - 12)
    return off, 128


def _groups(gq):
    """Pack this gq's k-tiles into groups of <=GROUP with bank-aligned
    positions (no matmul output crosses a 512-col PSUM bank boundary).
    Returns list of [(kp, pos, off, w), ...] per group."""
    kps = list(range(4 * (gq + 1))) if gq < 3 else list(range(16))
    out = []
    cur = []
    pos = 0
    for kp in kps:
        off, w = _geom(kp, gq)
        # bank-align: place within current 512 bank if it fits
        bank_rem = -pos % 512
        if bank_rem and w > bank_rem:
            pos += bank_rem
        if len(cur) == GROUP or pos + w > SPW:
            out.append(cur)
            cur = []
            pos = 0
        cur.append((kp, pos, off, w))
        pos += w
    if cur:
        out.append(cur)
    return out


def _build_program():
    import os
    import concourse.bass as bass
    import concourse.mybir as mybir
    import concourse.tile as tile

    rowtile = bool(int(os.environ.get("K_ROWTILE", "1")))
    scalar_dma = bool(int(os.environ.get("K_SCALAR_DMA", "1")))

    f32 = mybir.dt.float32
    fp16 = mybir.dt.float16
    Exp = mybir.ActivationFunctionType.Exp

    nc = bass.Bass()

    kqv_d = nc.dram_tensor("kqv", [HPC, 128, KQV], fp16, kind="ExternalInput")
    mtt_d = nc.dram_tensor("mtt", [128, 12 * 512], fp16, kind="ExternalInput")
    msk_d = nc.dram_tensor("msk", [128, 256], fp16, kind="ExternalInput")
    ot_d = nc.dram_tensor("ot", [HPC, 65, L], fp16, kind="ExternalOutput")

    with tile.TileContext(nc) as tc:
        with (
            tc.tile_pool(name="consts", bufs=1) as consts,
            tc.tile_pool(name="heads", bufs=4) as heads,
            tc.tile_pool(name="ptp", bufs=4) as ptp,
            tc.tile_pool(name="osbp", bufs=3) as osbp,
            tc.tile_pool(name="spp", bufs=2, space="PSUM") as spp,
            tc.tile_pool(name="avp", bufs=2, space="PSUM") as avp,
        ):
            # ---- input DMAs, all issued up front --------------------------
            msk_sb = consts.tile([128, 256], fp16)
            mtt_sb = consts.tile([128, 12 * 512], fp16)
            kqv_sbs = []
            for h in range(HPC):
                kqv_sbs.append(
                    heads.tile([128, KQV], fp16, tag="kqv", name=f"kqv{h}")
                )

            # Inputs via SWDGE (gpsimd): each dma_start is spread across all
            # 16 SDMA engines, vs HWDGE which serializes one queue per DMA.
            # head 0 split so first S-matmul columns land early.
            eng2 = nc.scalar if scalar_dma else nc.gpsimd
            nc.gpsimd.dma_start(
                out=kqv_sbs[0][:, 0:1024], in_=kqv_d.ap()[0][:, 0:1024]
            )
            nc.gpsimd.dma_start(
                out=kqv_sbs[0][:, QOFF : QOFF + 1024],
                in_=kqv_d.ap()[0][:, QOFF : QOFF + 1024],
            )
            eng2.dma_start(out=msk_sb, in_=msk_d.ap())
            nc.gpsimd.dma_start(
                out=kqv_sbs[0][:, 1024:2048], in_=kqv_d.ap()[0][:, 1024:2048]
            )
            nc.gpsimd.dma_start(
                out=kqv_sbs[0][:, QOFF + 1024 : VOFF],
                in_=kqv_d.ap()[0][:, QOFF + 1024 : VOFF],
            )
            nc.gpsimd.dma_start(
                out=kqv_sbs[0][:, VOFF:KQV], in_=kqv_d.ap()[0][:, VOFF:KQV]
            )
            nc.gpsimd.dma_start(out=mtt_sb, in_=mtt_d.ap())
            nc.gpsimd.dma_start(out=kqv_sbs[1], in_=kqv_d.ap()[1])
            nc.gpsimd.dma_start(out=kqv_sbs[2], in_=kqv_d.ap()[2])
            nc.gpsimd.dma_start(out=kqv_sbs[3], in_=kqv_d.ap()[3])

            mdiag = msk_sb[:, 0:128]
            mchunk = msk_sb[:, 128:256]

            s_idx = 0  # global S-matmul counter for PE row-half alternation
            osb_i = 0
            pending = []  # (av, kqv_sb, grp, pt, last_kp, fin) fin=(h,gq)|None

            def pop_one():
                nonlocal osb_i
                av_, kqv_, grp_, pt_, last_, fin_ = pending.pop(0)
                for kp, pos, off, w in grp_:
                    nc.tensor.matmul(
                        av_[:65, off : off + w],
                        lhsT=kqv_[:, VOFF + 65 * kp : VOFF + 65 * kp + 65],
                        rhs=pt_[:, pos : pos + w],
                        start=kp == 0,
                        stop=kp == last_,
                        skip_group_check=True,
                    )
                if fin_ is not None:
                    h_, gq_ = fin_
                    osb = osbp.tile([65, 512], fp16, name=f"osb{osb_i}")
                    osb_i += 1
                    nc.vector.tensor_copy(osb, av_[:65, :])
                    nc.gpsimd.dma_start(
                        out=ot_d.ap()[h_][:, 512 * gq_ : 512 * gq_ + 512],
                        in_=osb,
                    )

            for h in range(HPC):
                kqv_sb = kqv_sbs[h]
                for gq in range(4):
                    av = avp.tile([128, 512], f32, tag="av")
                    groups = _groups(gq)
                    last_kp = groups[-1][-1][0]

                    for gi, grp in enumerate(groups):
                        span = grp[-1][1] + grp[-1][3]
                        sp = spp.tile([128, SPW], f32, tag="sp")
                        for kp, pos, off, w in grp:
                            if rowtile:
                                half = (
                                    slice(0, 64) if s_idx % 2 == 0 else slice(64, 128)
                                )
                            else:
                                # K/Q rows are duplicated, so a full-128
                                # contraction computes 2*S; EXP scale halves.
                                half = slice(0, 128)
                            s_idx += 1
                            qs = QOFF + 512 * gq + off
                            nc.tensor.matmul(
                                sp[:, pos : pos + w],
                                lhsT=kqv_sb[half, 128 * kp : 128 * kp + 128],
                                rhs=kqv_sb[half, qs : qs + w],
                                start=True,
                                stop=True,
                                skip_group_check=True,
                            )
                        pt = ptp.tile([128, SPW], fp16, tag="pt")
                        nc.scalar.activation(
                            pt[:, 0:span],
                            sp[:, 0:span],
                            Exp,
                            scale=0.125 if rowtile else 0.0625,
                        )
                        if gq == 3 and grp[0][0] <= 11:
                            # per-batch test-train 0/1 mask on DVE; group spans
                            # mtt cols [512*kp0, 512*kp0 + span)
                            m0 = 512 * grp[0][0]
                            nc.vector.tensor_mul(
                                pt[:, 0:span],
                                pt[:, 0:span],
                                mtt_sb[:, m0 : m0 + span],
                            )
                        for kp, pos, off, w in grp:
                            if kp <= 11 and 128 * kp >= 512 * gq:
                                nc.vector.tensor_mul(
                                    pt[:, pos : pos + 128],
                                    pt[:, pos : pos + 128],
                                    mdiag,
                                )
                            elif kp >= 12:
                                nc.vector.tensor_mul(
                                    pt[:, pos : pos + 128],
                                    pt[:, pos : pos + 128],
                                    mchunk,
                                )

                        fin = (h, gq) if gi == len(groups) - 1 else None
                        pending.append((av, kqv_sb, grp, pt, last_kp, fin))
                        while len(pending) > PIPE:
                            pop_one()

            while pending:
                pop_one()

    import concourse.mybir as mybir_mod

    _split_multi_waits(nc, mybir_mod)
    return nc


def _host_inputs(queries, keys, values, attach):
    """Build per-core input maps (host-side layout prep)."""
    f16 = np.float16
    p = np.arange(128)
    f = np.arange(128)
    mdiag = np.where(f[None, :] >= p[:, None], 1.0, 0.0).astype(np.float32)
    mchunk = np.where(
        (p[:, None] // CH == f[None, :] // CH) & (p[:, None] <= f[None, :]),
        1.0,
        0.0,
    ).astype(np.float32)
    msk = np.concatenate([mdiag, mchunk], axis=1)  # [128, 256]

    in_maps = []
    for c in range(NCORES):
        b, g = divmod(c, 2)
        hs = slice(HPC * g, HPC * (g + 1))
        q = queries[b][:, hs, :]          # [L, 4, D]
        k = keys[b][:, hs, :]
        v = values[b][:, hs, :]
        qt = q.transpose(1, 2, 0)         # [4, 64, L]
        kt = k.transpose(1, 2, 0)
        vw = np.empty((HPC, L, 65), np.float32)
        vw[:, :, :64] = v.transpose(1, 0, 2)
        vw[:, :, 64] = 1.0
        # [4, L, 65] -> [4, 128, KT*65] with row p holding tile-chunks
        vw = np.ascontiguousarray(
            vw.reshape(HPC, KT, 128, 65).transpose(0, 2, 1, 3).reshape(HPC, 128, KT * 65)
        )
        # combined [4, 128, KQV]: kt | qt | vw, with K/Q duplicated into
        # partitions 64-127 for PE row-tiling
        kqv = np.empty((HPC, 128, KQV), np.float32)
        kqv[:, :64, 0:L] = kt
        kqv[:, 64:, 0:L] = kt
        kqv[:, :64, QOFF:VOFF] = qt
        kqv[:, 64:, QOFF:VOFF] = qt
        kqv[:, :, VOFF:] = vw
        kg = (np.arange(12)[:, None] * 128 + np.arange(128)[None, :])  # [12,128]
        thr = attach[b][np.arange(TEST) // CH]                          # [512]
        mtt = np.where(kg[:, :, None] <= thr[None, None, :], 1.0, 0.0)  # [12,128,512]
        mtt = np.ascontiguousarray(mtt.transpose(1, 0, 2).reshape(128, 12 * 512))
        in_maps.append(
            {
                "kqv": kqv.astype(f16),
                "mtt": mtt.astype(f16),
                "msk": msk.astype(f16),
            }
        )
    return in_maps


def kernel(queries, keys, values, attach_test_after, train_len):
    global LAST_RESULT, _PROG
    import os

    queries = np.asarray(queries, dtype=np.float32)
    keys = np.asarray(keys, dtype=np.float32)
    values = np.asarray(values, dtype=np.float32)
    attach = np.asarray(attach_test_after).astype(np.int64)
    tl = int(np.asarray(train_len))
    assert queries.shape == (B, L, H, D), queries.shape
    assert tl == TRAIN and attach.shape == (B, NCH)

    from concourse.bass_utils import run_bass_kernel_spmd

    if _PROG is None:
        _PROG = _build_program()

    in_maps = _host_inputs(queries, keys, values, attach)
    trace = bool(int(os.environ.get("KERNEL_TRACE", "0")))
    res = run_bass_kernel_spmd(
        _PROG, in_maps, core_ids=list(range(NCORES)), trace=trace
    )
    LAST_RESULT = res

    out = np.empty((B, L, H * D), np.float32)
    for c in range(NCORES):
        b, g = divmod(c, 2)
        ot = res.results[c]["ot"].astype(np.float32)  # [4, 65, L] (fp16 on dev)
        o = ot[:, :64, :] / ot[:, 64:65, :]           # [4, 64, L]
        out[b, :, 256 * g : 256 * (g + 1)] = (
            o.transpose(2, 0, 1).reshape(L, HPC * D)
        )
    return out
